# Initial kernel scaffold
#
"""CP-decomposed 3x3 conv on 8 TRN2 NeuronCores.

Math: out[f,i,j] = sum_{h,w,c,r} in[c,i+h,j+w] * f1[h,r] * f2[w,r] * f3[c,r] * f0[f,r]

Factorization used on-device (per core, over its slice of output rows):
  stage A: t2[r, n]  = sum_h sum_c (f3[c,r]*f1[h,r]) * x[c, n + h*W]     (3 matmuls, K=C)
  stage B: out[f, n] = sum_w sum_r (f2[w,r]*f0[f,r]) * t2[r, n + w]      (3 matmuls, K=R)
where n flattens (row, col) with row pitch W=256; output cols 254/255 of each
row are garbage and are skipped by the output DMA.

Sharding: output rows (Ho=254) split across 8 cores: cores 0-6 get rows
[32i, 32i+32); core 7 processes rows [222, 254) via a shifted window (its
first 2 rows duplicate core 6's tail and are dropped at gather).
"""

import sys

sys.path.insert(0, "/opt/trn_rl_repo")

import numpy as np

# Problem constants (hardcoded per contract)
C = 64
H = 256
W = 256
FH = 3
FW = 3
RANK = 64
F = 128
HO = H - FH + 1  # 254
WO = W - FW + 1  # 254
NCORES = 8
ROWS = 32  # output rows per core
IN_ROWS = ROWS + 2
CHUNK = 512  # output elements per matmul (= 2 rows x 256), one PSUM bank

# Compute dtype for matmul operands: "fp32" | "bf16"
COMPUTE_DT = "bf16"
# Chunk grouping: False (simple), True (paired), or "quad"
PAIRED = "quad"
# Ablation switches for benchmarking: subset of
# {"in_dma", "out_dma", "stage_a", "stage_b", "copies", "all"}
ABLATE = set()
# Engines for the two per-quad PSUM->SBUF output copies
COPY3_ENGINES = ("vector", "scalar")

_PROGRAM_CACHE = {}


def _np_compute_dtype():
    if COMPUTE_DT == "bf16":
        import ml_dtypes

        return np.dtype(ml_dtypes.bfloat16)
    return np.dtype(np.float32)


def build_program(
    rows=ROWS,
    compute_dt=None,
    num_devices=NCORES,
    reps=1,
    paired=None,
    bench_internal=False,
):
    """Build + compile the per-core Bass program. rows must be even.

    reps>1 wraps the whole body in a device-side loop (benchmarking only).
    paired=True processes chunks two at a time on disjoint halves of the PE
    array (col-split for stage A, row-split for stage B) so their matmuls
    run concurrently.
    """
    from concourse import bacc, mybir, tile
    from contextlib import ExitStack

    compute_dt = compute_dt or COMPUTE_DT
    if paired is None:
        paired = PAIRED
    if compute_dt == "bf16":
        dt_c = mybir.dt.bfloat16
    elif compute_dt == "fp32r":
        dt_c = mybir.dt.float32r
    else:
        dt_c = mybir.dt.float32
    dt_f32 = mybir.dt.float32

    in_rows = rows + 2
    nchunk = rows // 2

    nc = bacc.Bacc(
        "TRN2", target_bir_lowering=False, debug=False, num_devices=num_devices
    )
    if bench_internal:
        # Pure device-time benchmarking: all real I/O on internal DRAM
        # scratch so the host transfer per call is tiny.
        x = nc.dram_tensor("x_int", [C, in_rows, W], dt_f32).ap()
        wa = nc.dram_tensor("wa_int", [FH, C, RANK], dt_c).ap()
        wb = nc.dram_tensor("wb_int", [FW, RANK, F], dt_c).ap()
        y = nc.dram_tensor("y_int", [F, rows, WO], dt_f32).ap()
        tin = nc.dram_tensor("tin", [1, 16], dt_f32, kind="ExternalInput").ap()
        tout = nc.dram_tensor("tout", [1, 16], dt_f32, kind="ExternalOutput").ap()
    else:
        x = nc.dram_tensor("x", [C, in_rows, W], dt_f32, kind="ExternalInput").ap()
        wa = nc.dram_tensor("wa", [FH, C, RANK], dt_c, kind="ExternalInput").ap()
        wb = nc.dram_tensor("wb", [FW, RANK, F], dt_c, kind="ExternalInput").ap()
        y = nc.dram_tensor("y", [F, rows, WO], dt_f32, kind="ExternalOutput").ap()

    with tile.TileContext(nc) as tc:
        with (
            tc.tile_pool(name="xin", bufs=1) as xin_pool,
            tc.tile_pool(name="wgt", bufs=1) as wgt_pool,
            tc.tile_pool(name="t2", bufs=3) as t2_pool,
            tc.tile_pool(name="ot", bufs=3) as ot_pool,
            tc.tile_pool(name="p1", bufs=2, space="PSUM") as p1_pool,
            tc.tile_pool(
                name="p2", bufs=(2 if paired == "quad" else 4), space="PSUM"
            ) as p2_pool,
        ):

            def load_common():
                X = xin_pool.tile([C, in_rows * W], dt_c)
                WA = wgt_pool.tile([C, FH * RANK], dt_c, tag="wa")
                nc.sync.dma_start(
                    out=WA.rearrange("c (h r) -> c h r", r=RANK),
                    in_=wa.rearrange("h c r -> c h r"),
                )
                # Input load, split into 4 DMAs (queue parallelism).
                if "in_dma" in ABLATE:
                    nc.vector.memset(X[:, 0:8], 0.0)
                elif True:
                    xflat = x.rearrange("c h w -> c (h w)")
                    n_split = 4 if in_rows >= 8 else 1
                    bnds = [round(i * in_rows / n_split) for i in range(n_split + 1)]
                    dma_eng = nc.gpsimd if dt_c != dt_f32 else nc.sync
                    for a, b in zip(bnds, bnds[1:]):
                        dma_eng.dma_start(
                            out=X[:, a * W : b * W], in_=xflat[:, a * W : b * W]
                        )
                if bench_internal:
                    nc.sync.dma_start(out=tout[:], in_=tin[:])
                return X, WA

            def store_chunk(m, p2):
                ot = ot_pool.tile([F, CHUNK], dt_f32)
                if m % 2 == 0:
                    nc.scalar.copy(out=ot[:], in_=p2[:])
                else:
                    nc.vector.tensor_copy(out=ot[:], in_=p2[:])
                ov = ot.rearrange("f (r w) -> f r w", w=W)
                nc.sync.dma_start(out=y[:, 2 * m : 2 * m + 2, :], in_=ov[:, :, 0:WO])

            def body():
                if "all" in ABLATE:
                    junk = t2_pool.tile([RANK, CHUNK + 4], dt_c)
                    nc.vector.memset(junk[:, 0:8], 0.0)
                    if bench_internal:
                        nc.sync.dma_start(out=tout[:], in_=tin[:])
                    return
                X, WA = load_common()
                WB = wgt_pool.tile([RANK, FW * F], dt_c, tag="wb")
                nc.sync.dma_start(
                    out=WB.rearrange("r (w f) -> r w f", f=F),
                    in_=wb.rearrange("w r f -> r w f"),
                )

                for m in range(nchunk):
                    base = m * CHUNK
                    # Stage A: t2 = sum_h A_h^T @ x(shift h rows)
                    p1 = p1_pool.tile([C, CHUNK], dt_f32)
                    if "stage_a" in ABLATE:
                        nc.vector.memset(p1[:, 0:8], 0.0)
                    else:
                        for h in range(FH):
                            nc.tensor.matmul(
                                out=p1[:],
                                lhsT=WA[:, h * RANK : (h + 1) * RANK],
                                rhs=X[:, base + h * W : base + h * W + CHUNK],
                                start=(h == 0),
                                stop=(h == FH - 1),
                            )
                    # Evacuate PSUM -> SBUF (cast to compute dtype if needed).
                    # Width CHUNK+4 so stage-B shifted reads stay inside the
                    # tile; trailing elements only feed discarded columns.
                    t2 = t2_pool.tile([RANK, CHUNK + 4], dt_c)
                    if "copies" in ABLATE:
                        nc.vector.memset(t2[:, 0:8], 0.0)
                    else:
                        nc.vector.tensor_copy(out=t2[:, 0:CHUNK], in_=p1[:])
                        nc.vector.memset(t2[:, CHUNK : CHUNK + 4], 0.0)
                    # Stage B: out = sum_w B_w^T @ t2(shift w)
                    p2 = p2_pool.tile([F, CHUNK], dt_f32)
                    if "stage_b" in ABLATE:
                        nc.vector.memset(p2[:, 0:8], 0.0)
                    else:
                        for w in range(FW):
                            nc.tensor.matmul(
                                out=p2[:],
                                lhsT=WB[:, w * F : (w + 1) * F],
                                rhs=t2[:, w : w + CHUNK],
                                start=(w == 0),
                                stop=(w == FW - 1),
                            )
                    if "out_dma" not in ABLATE:
                        store_chunk(m, p2)

            def body_paired():
                X, WA = load_common()
                # WB duplicated into both partition halves so stage-B matmuls
                # for the two paired chunks run on disjoint PE row groups.
                WB2 = wgt_pool.tile([2 * RANK, FW * F], dt_c, tag="wb")
                for half in range(2):
                    nc.sync.dma_start(
                        out=WB2.rearrange("r (w f) -> r w f", f=F)[
                            half * RANK : (half + 1) * RANK
                        ],
                        in_=wb.rearrange("w r f -> r w f"),
                    )

                npair = nchunk // 2
                pending = None  # (m0, p2a, p2b) awaiting store
                for pi in range(npair + 1):
                    if pi < npair:
                        m0, m1 = 2 * pi, 2 * pi + 1
                        b0, b1 = m0 * CHUNK, m1 * CHUNK
                        # Stage A: chunk m0 -> PSUM cols 0-63, m1 -> cols 64-127
                        p1 = p1_pool.tile([2 * C, CHUNK], dt_f32)
                        for h in range(FH):
                            for k, bb in ((0, b0), (1, b1)):
                                nc.tensor.matmul(
                                    out=p1[k * C : (k + 1) * C, :],
                                    lhsT=WA[:, h * RANK : (h + 1) * RANK],
                                    rhs=X[:, bb + h * W : bb + h * W + CHUNK],
                                    start=(h == 0),
                                    stop=(h == FH - 1),
                                    # The two col-halves run interleaved
                                    # accumulation groups on one bank;
                                    # per-partition-slice clears are safe.
                                    skip_group_check=True,
                                )
                        t2 = t2_pool.tile([2 * RANK, CHUNK + 4], dt_c)
                        nc.vector.tensor_copy(out=t2[:, 0:CHUNK], in_=p1[:])
                        nc.vector.memset(t2[:, CHUNK : CHUNK + 4], 0.0)
                        # Stage B on disjoint row groups (rhs partitions 0-63
                        # for m0, 64-127 for m1), separate PSUM banks.
                        p2a = p2_pool.tile([F, CHUNK], dt_f32, tag="p2")
                        p2b = p2_pool.tile([F, CHUNK], dt_f32, tag="p2")
                        for w in range(FW):
                            for k, p2 in ((0, p2a), (1, p2b)):
                                nc.tensor.matmul(
                                    out=p2[:],
                                    lhsT=WB2[
                                        k * RANK : (k + 1) * RANK,
                                        w * F : (w + 1) * F,
                                    ],
                                    rhs=t2[k * RANK : (k + 1) * RANK, w : w + CHUNK],
                                    start=(w == 0),
                                    stop=(w == FW - 1),
                                )
                        new_pending = (m0, p2a, p2b)
                    else:
                        new_pending = None
                    # Store the previous pair (software-pipelined by one pair
                    # so PE never waits on the PSUM evacuations).
                    if pending is not None:
                        pm0, pa, pb = pending
                        store_chunk(pm0, pa)
                        store_chunk(pm0 + 1, pb)
                    pending = new_pending

            def body_quad():
                # 4 chunks (8 output rows) per quad iteration:
                #  - X and WA duplicated into both partition halves so the two
                #    stage-A pair-members occupy fully disjoint PE quadrants
                #    (rows AND cols) -> LDWEIGHTS + MATMUL run concurrently
                #  - stage A packs the 4 chunks as 2 quadrants x 2 banks in
                #    one (128, 1024) PSUM tile -> ONE copy to SBUF
                #  - t2 layout: partition half k holds the contiguous row
                #    stream of chunks (4q+2k, 4q+2k+1)
                #  - stage B: 2 row-groups x 2 banks into two (128, 1024)
                #    PSUM tiles -> one evacuation + one 4-row DMA each
                #  - stage B runs one quad behind stage A (software pipeline)
                X2 = xin_pool.tile([2 * C, in_rows * W], dt_c)
                WA2 = wgt_pool.tile([2 * C, FH * RANK], dt_c, tag="wa")
                WB2 = wgt_pool.tile([2 * RANK, FW * F], dt_c, tag="wb")
                for half in range(2):
                    nc.sync.dma_start(
                        out=WA2.rearrange("c (h r) -> c h r", r=RANK)[
                            half * C : (half + 1) * C
                        ],
                        in_=wa.rearrange("h c r -> c h r"),
                    )
                    nc.sync.dma_start(
                        out=WB2.rearrange("r (w f) -> r w f", f=F)[
                            half * RANK : (half + 1) * RANK
                        ],
                        in_=wb.rearrange("w r f -> r w f"),
                    )
                if "in_dma" in ABLATE:
                    nc.vector.memset(X2[:, 0:8], 0.0)
                else:
                    xflat = x.rearrange("c h w -> c (h w)")
                    half_rows = (in_rows + 1) // 2
                    dma_eng = nc.gpsimd if dt_c != dt_f32 else nc.sync
                    for half in range(2):
                        for a, b in ((0, half_rows), (half_rows, in_rows)):
                            dma_eng.dma_start(
                                out=X2[half * C : (half + 1) * C, a * W : b * W],
                                in_=xflat[:, a * W : b * W],
                            )
                if bench_internal:
                    nc.sync.dma_start(out=tout[:], in_=tin[:])

                def stage_a(q):
                    # (half k, slot g) -> chunk 4q + 2k + g
                    p1q = p1_pool.tile([2 * C, 2 * CHUNK], dt_f32)
                    if "stage_a" in ABLATE:
                        nc.vector.memset(p1q[:, 0:8], 0.0)
                    else:
                        for h in range(FH):
                            for k, g in ((0, 0), (1, 0), (0, 1), (1, 1)):
                                m = 4 * q + 2 * k + g
                                bb = m * CHUNK
                                nc.tensor.matmul(
                                    out=p1q[
                                        k * C : (k + 1) * C, g * CHUNK : (g + 1) * CHUNK
                                    ],
                                    lhsT=WA2[
                                        k * C : (k + 1) * C, h * RANK : (h + 1) * RANK
                                    ],
                                    rhs=X2[
                                        k * C : (k + 1) * C,
                                        bb + h * W : bb + h * W + CHUNK,
                                    ],
                                    start=(h == 0),
                                    stop=(h == FH - 1),
                                    skip_group_check=True,
                                )
                    t2q = t2_pool.tile([2 * RANK, 2 * CHUNK + 4], dt_c, tag="t2")
                    if "copies" in ABLATE:
                        nc.vector.memset(t2q[:, 0:8], 0.0)
                    else:
                        nc.vector.tensor_copy(out=t2q[:, 0 : 2 * CHUNK], in_=p1q[:])
                        nc.vector.memset(t2q[:, 2 * CHUNK : 2 * CHUNK + 4], 0.0)
                    return t2q

                def stage_b(q, t2q):
                    p2q0 = p2_pool.tile([F, 2 * CHUNK], dt_f32, tag="p2")
                    p2q1 = p2_pool.tile([F, 2 * CHUNK], dt_f32, tag="p2")
                    p2q = [p2q0, p2q1]
                    if "stage_b" in ABLATE:
                        nc.vector.memset(p2q0[:, 0:8], 0.0)
                        nc.vector.memset(p2q1[:, 0:8], 0.0)
                    else:
                        for w in range(FW):
                            for k, g in ((0, 0), (1, 0), (0, 1), (1, 1)):
                                nc.tensor.matmul(
                                    out=p2q[k][:, g * CHUNK : (g + 1) * CHUNK],
                                    lhsT=WB2[
                                        k * RANK : (k + 1) * RANK, w * F : (w + 1) * F
                                    ],
                                    rhs=t2q[
                                        k * RANK : (k + 1) * RANK,
                                        g * CHUNK + w : g * CHUNK + w + CHUNK,
                                    ],
                                    start=(w == 0),
                                    stop=(w == FW - 1),
                                    skip_group_check=True,
                                )
                    if "out_dma" not in ABLATE:
                        for k in range(2):
                            ot = ot_pool.tile([F, 2 * CHUNK], dt_f32)
                            eng = COPY3_ENGINES[k]
                            if eng == "vector":
                                nc.vector.tensor_copy(out=ot[:], in_=p2q[k][:])
                            else:
                                nc.scalar.copy(out=ot[:], in_=p2q[k][:])
                            ov = ot.rearrange("f (r w) -> f r w", w=W)
                            r0 = 8 * q + 4 * k
                            nc.sync.dma_start(
                                out=y[:, r0 : r0 + 4, :], in_=ov[:, :, 0:WO]
                            )

                nquad = nchunk // 4
                pending = None
                for q in range(nquad + 1):
                    t2q = stage_a(q) if q < nquad else None
                    if pending is not None:
                        stage_b(q - 1, pending)
                    pending = t2q

            if paired == "quad":
                body_fn = body_quad
            elif paired:
                body_fn = body_paired
            else:
                body_fn = body
            if reps == 1:
                body_fn()
            else:
                with tc.For_i(0, reps, 1):
                    body_fn()

    nc.compile()
    return nc


def _get_program():
    key = (ROWS, COMPUTE_DT)
    if key not in _PROGRAM_CACHE:
        _PROGRAM_CACHE[key] = build_program()
    return _PROGRAM_CACHE[key]


def make_weight_inputs(factor0, factor1, factor2, factor3, np_dt=None):
    np_dt = np_dt or _np_compute_dtype()
    f0 = np.asarray(factor0, np.float32)
    f1 = np.asarray(factor1, np.float32)
    f2 = np.asarray(factor2, np.float32)
    f3 = np.asarray(factor3, np.float32)
    # wa[h,c,r] = f3[c,r] * f1[h,r]
    wa = (f3[None, :, :] * f1[:, None, :]).astype(np_dt)
    # wb[w,r,f] = f2[w,r] * f0[f,r]
    wb = (f2[:, :, None] * f0.T[None, :, :]).astype(np_dt)
    return wa, wb


ROW_STARTS = [0, 32, 64, 96, 128, 160, 192, 222]


def kernel(input, factor0, factor1, factor2, factor3):
    from concourse.bass_utils import run_bass_kernel_spmd

    nc = _get_program()
    wa, wb = make_weight_inputs(factor0, factor1, factor2, factor3)
    inp = np.ascontiguousarray(np.asarray(input, np.float32))
    in_maps = [
        {
            "x": np.ascontiguousarray(inp[:, s : s + IN_ROWS, :]),
            "wa": wa,
            "wb": wb,
        }
        for s in ROW_STARTS
    ]
    res = run_bass_kernel_spmd(nc, in_maps, list(range(NCORES))).results
    out = np.empty((F, HO, WO), np.float32)
    for i, s in enumerate(ROW_STARTS):
        ys = res[i]["y"]
        if i < NCORES - 1:
            out[:, s : s + ROWS, :] = ys
        else:
            out[:, 224:HO, :] = ys[:, 2:ROWS, :]
    return out



# revision 3
# speedup vs baseline: 1.6118x; 1.6118x over previous
"""CP-decomposed 3x3 conv on 8 TRN2 NeuronCores.

Math: out[f,i,j] = sum_{h,w,c,r} in[c,i+h,j+w] * f1[h,r] * f2[w,r] * f3[c,r] * f0[f,r]

Factorization used on-device (per core, over its 32-row slice of output):
  stage A: t2[r, n]  = sum_h sum_c (f3[c,r]*f1[h,r]) * x[c, n + h*W]     (3 matmuls, K=C)
  stage B: out[f, n] = sum_w sum_r (f2[w,r]*f0[f,r]) * t2[r, n + w]      (3 matmuls, K=R)
where n flattens (row, col) with row pitch W=256; output cols 254/255 of each
row are garbage (the host slices them off).

Per-core layout (all fp16 on device, fp32 PSUM accumulation):
  - SBUF partition half 0 holds input rows [0,18) of the core's 34-row
    window, half 1 holds rows [16,34) -- a block split, NOT a duplicate, so
    the input is read from HBM once.
  - Each quad iteration computes 4 chunks (a chunk = 512 output positions =
    2 output rows): A,B = chunks 2q,2q+1 (from half 0) and C,D = chunks
    8+2q,8+2q+1 (from half 1).
  - Stage A runs the 4 chunks on 4 disjoint 64x64 PE quadrants concurrently
    (A=(0,0), B=(0,64), C=(64,0), D=(64,64) as (row,col) tile positions,
    auto-derived from AP base partitions).  t2 partition half 0 <- {A,C},
    half 1 <- {B,D}.  Chunks need not be adjacent in t2: the w-shift reads
    that cross a chunk boundary only feed output cols 254/255, which are
    dropped.
  - Stage B runs 2-wide (row-split k=0/1 over t2 partition halves), two
    serial bank slots g, filling the whole PE array.
  - PSUM evacuation is split between vector and scalar engines; outputs are
    cast to fp16 in the copy and DMAed (full 256-col rows for >=512B
    descriptor runs).

Sharding: output rows (Ho=254) split across 8 cores: cores 0-6 get rows
[32i, 32i+32); core 7 processes rows [222, 254) via a shifted window (its
first 2 rows duplicate core 6's tail and are dropped at gather).
"""

import sys

sys.path.insert(0, "/opt/trn_rl_repo")

import numpy as np

# Problem constants (hardcoded per contract)
C = 64
H = 256
W = 256
FH = 3
FW = 3
RANK = 64
F = 128
HO = H - FH + 1  # 254
WO = W - FW + 1  # 254
NCORES = 8
ROWS = 32  # output rows per core
IN_ROWS = ROWS + 2
HALF_OUT_ROWS = ROWS // 2  # 16
HALF_IN_ROWS = HALF_OUT_ROWS + 2  # 18
XCOLS = HALF_IN_ROWS * W  # columns of each X2 partition half
CHUNK = 512  # output positions per matmul (= 2 rows x 256), one PSUM bank
NCHUNK = ROWS // 2  # 16
NQUAD = NCHUNK // 4  # 4

# Compute dtype for matmul operands: "fp16" | "bf16" | "fp32"
COMPUTE_DT = "fp16"
# Output DRAM dtype: "fp16" | "fp32"
OUT_DT = "fp16"
# Ablation switches for benchmarking: subset of
# {"in_dma", "out_dma", "stage_a", "stage_b", "copies", "all"}
ABLATE = set()

_PROGRAM_CACHE = {}


def _np_dt(name):
    if name == "fp16":
        return np.dtype(np.float16)
    if name == "bf16":
        import ml_dtypes

        return np.dtype(ml_dtypes.bfloat16)
    return np.dtype(np.float32)


def build_program(
    compute_dt=None,
    out_dt=None,
    num_devices=NCORES,
    reps=1,
    bench_internal=False,
):
    """Build + compile the per-core Bass program.

    reps>1 wraps the body in a device-side hardware loop (benchmarking only).
    bench_internal puts the real I/O on internal DRAM scratch so the host
    transfer per call is tiny (timing runs only).
    """
    from concourse import bacc, mybir, tile

    compute_dt = compute_dt or COMPUTE_DT
    out_dt = out_dt or OUT_DT
    dt_map = {
        "fp16": mybir.dt.float16,
        "bf16": mybir.dt.bfloat16,
        "fp32": mybir.dt.float32,
    }
    dt_c = dt_map[compute_dt]
    dt_o = dt_map[out_dt]
    dt_f32 = mybir.dt.float32

    nc = bacc.Bacc(
        "TRN2", target_bir_lowering=False, debug=False, num_devices=num_devices
    )
    if bench_internal:
        x = nc.dram_tensor("x_int", [C, IN_ROWS, W], dt_c).ap()
        wa = nc.dram_tensor("wa_int", [C, FH * RANK], dt_c).ap()
        wb = nc.dram_tensor("wb_int", [RANK, FW * F], dt_c).ap()
        y = nc.dram_tensor("y_int", [F, ROWS, W], dt_o).ap()
        tin = nc.dram_tensor("tin", [1, 16], dt_f32, kind="ExternalInput").ap()
        tout = nc.dram_tensor("tout", [1, 16], dt_f32, kind="ExternalOutput").ap()
    else:
        x = nc.dram_tensor("x", [C, IN_ROWS, W], dt_c, kind="ExternalInput").ap()
        wa = nc.dram_tensor("wa", [C, FH * RANK], dt_c, kind="ExternalInput").ap()
        wb = nc.dram_tensor("wb", [RANK, FW * F], dt_c, kind="ExternalInput").ap()
        y = nc.dram_tensor("y", [F, ROWS, W], dt_o, kind="ExternalOutput").ap()

    with tile.TileContext(nc) as tc:
        with (
            tc.tile_pool(name="xin", bufs=1) as xin_pool,
            tc.tile_pool(name="wgt", bufs=1) as wgt_pool,
            tc.tile_pool(name="t2", bufs=3) as t2_pool,
            tc.tile_pool(name="ot", bufs=4) as ot_pool,
            tc.tile_pool(name="p1", bufs=2, space="PSUM") as p1_pool,
            tc.tile_pool(name="p2", bufs=2, space="PSUM") as p2_pool,
        ):
            if bench_internal:
                nc.sync.dma_start(out=tout[:], in_=tin[:])

            def body():
                X2 = xin_pool.tile([2 * C, XCOLS], dt_c)
                WA2 = wgt_pool.tile([2 * C, FH * RANK], dt_c, tag="wa")
                WB2 = wgt_pool.tile([2 * RANK, FW * F], dt_c, tag="wb")
                for half in range(2):
                    nc.sync.dma_start(
                        out=WA2[half * C : (half + 1) * C, :], in_=wa[:, :]
                    )
                    nc.sync.dma_start(
                        out=WB2[half * RANK : (half + 1) * RANK, :], in_=wb[:, :]
                    )
                if "in_dma" in ABLATE:
                    nc.vector.memset(X2[:, 0:8], 0.0)
                else:
                    xf = x.rearrange("c h w -> c (h w)")
                    for half in range(2):
                        r0 = HALF_OUT_ROWS * half
                        # Early rows first so quad 0 can start sooner.
                        for a, b in ((0, 7), (7, HALF_IN_ROWS)):
                            nc.sync.dma_start(
                                out=X2[half * C : (half + 1) * C, a * W : b * W],
                                in_=xf[:, (r0 + a) * W : (r0 + b) * W],
                            )

                def stage_a(q):
                    # 4 chunks on 4 disjoint PE quadrants per tap.
                    # (rhs half, psum col half, bank slot):
                    #   A=(0,0,0) B=(0,1,0) C=(1,0,1) D=(1,1,1)
                    p1q = p1_pool.tile([2 * C, 2 * CHUNK], dt_f32)
                    if "stage_a" in ABLATE:
                        nc.vector.memset(p1q[:, 0:8], 0.0)
                    else:
                        for t in range(FH):
                            for rh, ch, g in (
                                (0, 0, 0),
                                (0, 1, 0),
                                (1, 0, 1),
                                (1, 1, 1),
                            ):
                                # chunk local index j = 2q + ch; col offset
                                # (2j + t) * W
                                col = (4 * q + 2 * ch + t) * W
                                nc.tensor.matmul(
                                    out=p1q[
                                        ch * C : ch * C + C,
                                        g * CHUNK : (g + 1) * CHUNK,
                                    ],
                                    lhsT=WA2[
                                        rh * C : (rh + 1) * C,
                                        t * RANK : (t + 1) * RANK,
                                    ],
                                    rhs=X2[
                                        rh * C : (rh + 1) * C, col : col + CHUNK
                                    ],
                                    start=(t == 0),
                                    stop=(t == FH - 1),
                                    skip_group_check=True,
                                )
                    t2q = t2_pool.tile([2 * RANK, 2 * CHUNK + 4], dt_c, tag="t2")
                    nc.vector.memset(t2q[:, 2 * CHUNK : 2 * CHUNK + 4], 0.0)
                    if "copies" in ABLATE:
                        nc.vector.memset(t2q[:, 0:8], 0.0)
                    else:
                        nc.vector.tensor_copy(
                            out=t2q[:, 0:CHUNK], in_=p1q[:, 0:CHUNK]
                        )
                        nc.scalar.copy(
                            out=t2q[:, CHUNK : 2 * CHUNK],
                            in_=p1q[:, CHUNK : 2 * CHUNK],
                        )
                    return t2q

                def stage_b(q, t2q):
                    p2k0 = p2_pool.tile([F, 2 * CHUNK], dt_f32, tag="p2")
                    p2k1 = p2_pool.tile([F, 2 * CHUNK], dt_f32, tag="p2")
                    p2k = (p2k0, p2k1)
                    if "stage_b" in ABLATE:
                        nc.vector.memset(p2k0[:, 0:8], 0.0)
                        nc.vector.memset(p2k1[:, 0:8], 0.0)
                    else:
                        for w in range(FW):
                            for k, g in ((0, 0), (1, 0), (0, 1), (1, 1)):
                                nc.tensor.matmul(
                                    out=p2k[k][:, g * CHUNK : (g + 1) * CHUNK],
                                    lhsT=WB2[
                                        k * RANK : (k + 1) * RANK,
                                        w * F : (w + 1) * F,
                                    ],
                                    rhs=t2q[
                                        k * RANK : (k + 1) * RANK,
                                        g * CHUNK + w : g * CHUNK + w + CHUNK,
                                    ],
                                    start=(w == 0),
                                    stop=(w == FW - 1),
                                    skip_group_check=True,
                                )
                    if "out_dma" in ABLATE:
                        return
                    ots = []
                    for k in range(2):
                        ot = ot_pool.tile([F, 2 * CHUNK], dt_o)
                        if "copies" in ABLATE:
                            nc.vector.memset(ot[:, 0:8], 0.0)
                        elif k == 0:
                            nc.vector.tensor_copy(out=ot[:], in_=p2k[k][:])
                        else:
                            nc.scalar.copy(out=ot[:], in_=p2k[k][:])
                        ots.append(ot)
                    # k tile holds chunks (2q+k | 8+2q+k): global output rows
                    # {4q+2k, 4q+2k+1} and {16+4q+2k, 16+4q+2k+1}.
                    yv = y.rearrange("f (b r) w -> f b r w", b=2)
                    for k in range(2):
                        ov = ots[k].rearrange("f (b r w) -> f b r w", b=2, w=W)
                        r0 = 4 * q + 2 * k
                        nc.sync.dma_start(
                            out=yv[:, :, r0 : r0 + 2, :], in_=ov[:]
                        )

                if "all" in ABLATE:
                    junk = t2_pool.tile([RANK, CHUNK], dt_c, tag="t2")
                    nc.vector.memset(junk[:, 0:8], 0.0)
                    return
                pending = None
                for q in range(NQUAD + 1):
                    t2q = stage_a(q) if q < NQUAD else None
                    if pending is not None:
                        stage_b(q - 1, pending)
                    pending = t2q

            if reps == 1:
                body()
            else:
                with tc.For_i(0, reps, 1):
                    body()

    nc.compile()
    return nc


def _get_program():
    key = (COMPUTE_DT, OUT_DT)
    if key not in _PROGRAM_CACHE:
        _PROGRAM_CACHE[key] = build_program()
    return _PROGRAM_CACHE[key]


def make_weight_inputs(factor0, factor1, factor2, factor3, np_dt=None):
    np_dt = np_dt or _np_dt(COMPUTE_DT)
    f0 = np.asarray(factor0, np.float32)
    f1 = np.asarray(factor1, np.float32)
    f2 = np.asarray(factor2, np.float32)
    f3 = np.asarray(factor3, np.float32)
    # wa[c, t*R + r] = f3[c,r] * f1[t,r]
    wa = (
        (f1[:, None, :] * f3[None, :, :]).transpose(1, 0, 2).reshape(C, FH * RANK)
    ).astype(np_dt)
    # wb[r, w*F + f] = f2[w,r] * f0[f,r]
    wb = (
        (f2[:, :, None] * f0.T[None, :, :]).transpose(1, 0, 2).reshape(RANK, FW * F)
    ).astype(np_dt)
    return np.ascontiguousarray(wa), np.ascontiguousarray(wb)


ROW_STARTS = [0, 32, 64, 96, 128, 160, 192, 222]


def make_in_maps(input, factor0, factor1, factor2, factor3):
    wa, wb = make_weight_inputs(factor0, factor1, factor2, factor3)
    np_dt = _np_dt(COMPUTE_DT)
    x16 = np.asarray(input).astype(np_dt)
    return [
        {
            "x": np.ascontiguousarray(x16[:, s : s + IN_ROWS, :]),
            "wa": wa,
            "wb": wb,
        }
        for s in ROW_STARTS
    ]


def kernel(input, factor0, factor1, factor2, factor3):
    from concourse.bass_utils import run_bass_kernel_spmd

    nc = _get_program()
    in_maps = make_in_maps(input, factor0, factor1, factor2, factor3)
    res = run_bass_kernel_spmd(nc, in_maps, list(range(NCORES))).results
    out = np.empty((F, HO, WO), np.float32)
    for i, s in enumerate(ROW_STARTS):
        ys = np.asarray(res[i]["y"])[:, :, 0:WO].astype(np.float32)
        if i < NCORES - 1:
            out[:, s : s + ROWS, :] = ys
        else:
            out[:, 224:HO, :] = ys[:, 2:ROWS, :]
    return out
